# revision 30
# baseline (speedup 1.0000x reference)
"""Fused dual-branch attention kernel for one TRN2 chip (8 NeuronCores).

Problem: x:[4,1024,1024], qkv_w:[3072,1024], proj_w:[1024,1024], proj_b:[1024],
attn_mask:[2,1,1024,1024].  Reference computes two attention branches sharing
the qkv/proj weights:
  x_ori = proj(attend(q, k, v, mask0)),  x_v = proj(attend(v, v, v, mask1))

Sharding: 8 cores = (2 branches x 4 batches), zero communication.  Every core
runs the SAME graph; branch differences are folded into the per-core weight
data (branch-1 cores get [v_w*s | v_w | v_w] as their "qkv" weight stack) and
the per-core mask data.  The softmax scale folds into the query weights; the
additive mask folds in multiplicatively as exp(mask) (no max-subtraction:
logits are bounded ~+-8 for this distribution, exp stays in fp32 range).

Device layouts (host pre-transposes; all matmul operands land naturally):
  xT   [C, N]      x[b]^T
  wT   [C, 3C]     [A|B|C] weight stack transposed (A = query role, scaled)
  em   [N, N]      exp(mask)^T  (indexed [m, nq])
  pwT  [C, C]      proj_w^T
  out  [C, N]      y^T (host transposes back)

Per-head attention with everything transposed:
  ST[m,nq]  = sum_d BT[d,m] * AT[d,nq]          (K=64 matmul, psum)
  PT[m,nq]  = exp(ST) * em[m,nq]                (ACT exp -> DVE mult, bf16)
  PV        = [V_h | ones]^T-style lhsT gives OT'[d,nq] rows 0..63 and the
              softmax denominator sum_m PT[m,nq] in row 64 of the same psum.
  OT[d,nq]  = OT'[d,nq] * recip(denom)[nq]      (recip bcast across partitions)
Then yT = pwT^T @ OT_all + b.
"""

import numpy as np
import ml_dtypes

import concourse.bass as bass
from concourse import bacc
import concourse.tile as tile
import concourse.mybir as mybir
from contextlib import ExitStack

B, N, C, H, D, P, NF = 4, 1024, 1024, 16, 64, 128, 512
BF16 = mybir.dt.bfloat16
F32 = mybir.dt.float32
AF = mybir.ActivationFunctionType

_nc_cache = None


def _build(reps=1):
    nc = bacc.Bacc("TRN2", target_bir_lowering=False, debug=False, num_devices=8)
    xT = nc.declare_dram_parameter("xT", [C, N], BF16, isOutput=False)
    wT = nc.declare_dram_parameter("wT", [C, 3 * C], BF16, isOutput=False)
    em = nc.declare_dram_parameter("em", [N, N], BF16, isOutput=False)
    pwT = nc.declare_dram_parameter("pwT", [C, C], BF16, isOutput=False)
    pb = nc.declare_dram_parameter("pb", [C], F32, isOutput=False)
    out = nc.declare_dram_parameter("out", [C, N], F32, isOutput=True)

    with tile.TileContext(nc) as tc:
        for _ in range(reps):
            with ExitStack() as ctx:
                _body(tc, ctx, xT, wT, em, pwT, pb, out)
    nc.compile()
    return nc


def _body(tc, ctx, xT, wT, em, pwT, pb, out):
    nc = tc.nc

    pers = ctx.enter_context(tc.tile_pool(name="pers", bufs=1))
    work = ctx.enter_context(tc.tile_pool(name="work", bufs=1))
    psum = ctx.enter_context(tc.tile_pool(name="psum", bufs=1, space="PSUM"))

    def ps_tile(nm):
        return psum.tile([P, N], F32, name=nm, tag="ps", bufs=2)

    # x tiles first (A2 needs them), then the C-role weight columns, then the
    # A/B-role weight columns, then everything else — DMA arrival order
    # matches compute order so PE starts as early as possible.
    xw = ctx.enter_context(tc.tile_pool(name="xw", bufs=1))
    x_t, w_t = [], []
    with tc.tile_pool(name="wc", bufs=1) as wcp:
        wc_t = []
        for c in range(8):
            x = xw.tile([P, N], BF16, name=f"x{c}", tag=f"x{c}")
            nc.sync.dma_start(x[:, 0:P], xT[c * P:(c + 1) * P, 0:P])
            x_t.append(x)
            w = wcp.tile([P, C], BF16, name=f"wc{c}", tag=f"wc{c}")
            nc.sync.dma_start(w[:, 0:NF], wT[c * P:(c + 1) * P, 2 * C:2 * C + NF])
            wc_t.append(w)
        for c in range(8):
            nc.sync.dma_start(x_t[c][:, P:NF], xT[c * P:(c + 1) * P, P:NF])
        for c in range(8):
            nc.sync.dma_start(x_t[c][:, NF:N], xT[c * P:(c + 1) * P, NF:N])
            nc.sync.dma_start(wc_t[c][:, NF:N], wT[c * P:(c + 1) * P, 2 * C + NF:3 * C])
        for c in range(8):
            w = xw.tile([P, 2 * C], BF16, name=f"w{c}", tag=f"w{c}")
            nc.sync.dma_start(w[:], wT[c * P:(c + 1) * P, 0:2 * C])
            w_t.append(w)
        em_t = []
        for m in range(8):
            t = pers.tile([P, N], BF16, name=f"em{m}", tag=f"em{m}")
            nc.sync.dma_start(t[:], em[m * P:(m + 1) * P, :])
            em_t.append(t)
        pb_t = pers.tile([P, 8], F32, name="pb", tag="pb")
        nc.sync.dma_start(pb_t[:], pb.rearrange("(t p) -> p t", p=P))

        ones_t = pers.tile([P, 64], BF16, name="ones", tag="ones")
        nc.vector.memset(ones_t[:], 1.0)
        abt = [pers.tile([P, N], BF16, name=f"abt{i}", tag=f"abt{i}") for i in range(16)]
        v65 = [pers.tile([P, 16 * 65], BF16, name=f"v65_{m}", tag=f"v65_{m}") for m in range(8)]

        # ===== A2 first: V in natural [n, c] layout (+ ones cols at stride 65).
        # vf-outer so the first sweep only needs the first wc halves; the m-th
        # group only needs columns m*128 of each x tile, matching DMA arrival.
        for m in range(8):
            nc.vector.memset(v65[m].rearrange("p (h e) -> p h e", e=65)[:, :, 64:65], 1.0)
        for vf in range(2):
            for m in range(8):
                ps = psum.tile([P, NF], F32, name=f"v{m}_{vf}", tag="qk", bufs=2)
                for c in range(8):
                    nc.tensor.matmul(
                        ps[:],
                        lhsT=x_t[c][:, m * P:(m + 1) * P],
                        rhs=wc_t[c][:, vf * NF:(vf + 1) * NF],
                        start=(c == 0), stop=(c == 7),
                    )
                v3d = v65[m].rearrange("p (h e) -> p h e", e=65)
                nc.vector.tensor_copy(v3d[:, vf * 8:(vf + 1) * 8, 0:64],
                                      ps.rearrange("p (h d) -> p h d", d=64))

    # pools first used after A2 can reuse the wc pool space
    late = ctx.enter_context(tc.tile_pool(name="late", bufs=1))
    pw_t = []
    for c in range(8):
        t = late.tile([P, C], BF16, name=f"pw{c}", tag=f"pw{c}")
        nc.sync.dma_start(t[:], pwT[c * P:(c + 1) * P, :])
        pw_t.append(t)
    ot = [late.tile([P, N], BF16, name=f"ot{i}", tag=f"ot{i}") for i in range(8)]

    # ===== Fused A1 + attention, modulo-scheduled =====
    # Steady state per m-step: PE issues ST(h,m) [2 mm], PV(h-1,m) [2 mm] and
    # every 4th step one A1 half-group [8 mm]; ACT runs exp(h,m); DVE runs the
    # exp-mask multiply, A1 evictions and the normalize chain of head h-1.
    # One-head software pipelining keeps every engine fed; A1 tiles complete
    # ~2 steps before their pair's STs need them.
    def a1_half(mt, nh):
        ps = psum.tile([P, NF], F32, name=f"qk{mt}_{nh}", tag="qk", bufs=2)
        for c in range(8):
            nc.tensor.matmul(
                ps[:],
                lhsT=w_t[c][:, mt * P:(mt + 1) * P],
                rhs=x_t[c][:, nh * NF:(nh + 1) * NF],
                start=(c == 0), stop=(c == 7),
            )
        nc.vector.tensor_copy(abt[mt][:, nh * NF:(nh + 1) * NF], ps[:])

    pts = [[None] * 8, [None] * 8]
    for mt in (0, 8, 1, 9):           # kickstart: q/k tiles for pairs 0 and 1
        for nh in range(2):
            a1_half(mt, nh)
    _a1 = [(8 * r + p, nh) for p in range(2, 8) for r in range(2) for nh in range(2)]
    # Emission step targets: as late as possible while completing pair p's
    # tiles well before its first head; spread ~5 steps apart so late heads
    # keep PE filler work.
    a1q = [(min(5 * i + 3, 16 * (2 + i // 4) - 10 + (i % 4) * 2), mt, nh)
           for i, (mt, nh) in enumerate(_a1)]
    a1q.reverse()                      # pop() takes the earliest target

    pv_halves = {}
    pv15 = None
    for h in range(16):
        # ST / exp / mult for head h, PV for head h-1, paced A1 prefetch
        if h < 16:
            tl, ro = h // 2, (h % 2) * 64
            q_ap, k_ap = abt[tl], abt[8 + tl]
        if h >= 1:
            hl = h - 1
            pvA = psum.tile([P, NF], F32, name=f"pv{hl}a", tag="pv", bufs=2)
            pvB = psum.tile([P, NF], F32, name=f"pv{hl}b", tag="pv", bufs=2)
            pv_halves[hl] = (pvA, pvB)
        for m in range(8):
            if h < 16:
                ps = psum.tile([P, N], F32, name=f"st{m}", tag="ps", bufs=2)
                for nh in range(2):
                    nc.tensor.matmul(
                        ps[:, nh * NF:(nh + 1) * NF],
                        lhsT=k_ap[ro:ro + 64, m * P:(m + 1) * P],
                        rhs=q_ap[ro:ro + 64, nh * NF:(nh + 1) * NF],
                        start=True, stop=True,
                    )
                e = work.tile([P, N], BF16, name="est", tag="est", bufs=3)
                nc.scalar.activation(e[:], ps[:], AF.Exp)
                pt = work.tile([P, N], BF16, name=f"pt{m}", tag=f"pt{m}", bufs=2)
                nc.vector.tensor_mul(pt[:], e[:], em_t[m][:])
                pts[h & 1][m] = pt
            if h >= 1:
                hl = h - 1
                for nh, pv in enumerate(pv_halves[hl]):
                    nc.tensor.matmul(
                        pv[0:65, :],
                        lhsT=v65[m][:, hl * 65:(hl + 1) * 65],
                        rhs=pts[hl & 1][m][:, nh * NF:(nh + 1) * NF],
                        start=(m == 0), stop=(m == 7),
                    )
            if h == 15 and m >= 1:
                # drain shortcut: head 15's PV runs with a 1-chunk lag inside
                # its own step (into a wide ps slot) so the final normalize
                # chain starts ~4us earlier.
                if pv15 is None:
                    pv15 = psum.tile([P, N], F32, name="pv15", tag="ps", bufs=2)
                for nh in range(2):
                    nc.tensor.matmul(
                        pv15[0:65, nh * NF:(nh + 1) * NF],
                        lhsT=v65[m - 1][:, 15 * 65:16 * 65],
                        rhs=pts[1][m - 1][:, nh * NF:(nh + 1) * NF],
                        start=(m == 1), stop=False,
                    )
            gstep = h * 8 + m
            if a1q and gstep >= a1q[-1][0]:
                _, mt, nh = a1q.pop()
                a1_half(mt, nh)
        if h >= 1:
            hl = h - 1
            tl, ro = hl // 2, (hl % 2) * 64
            rc = work.tile([P, N], BF16, name="rc", tag="rc", bufs=2)
            rc64 = work.tile([P, N], BF16, name="rc64", tag="rc64", bufs=2)
            og = (None if ro == 0 else
                  work.tile([64, N], BF16, name="otg", tag="otg", bufs=2))
            for nh, pv in enumerate(pv_halves[hl]):
                sl = slice(nh * NF, (nh + 1) * NF)
                with nc.allow_low_precision(reason="softmax denom bf16 ok"):
                    nc.vector.reciprocal(rc[64:65, sl], pv[64:65, :])
                # HW partition_broadcast reads absolute partition 0 of its
                # input (AP partition offsets are ignored, unlike CoreSim) —
                # DMA the row down to partition 0 first.
                rc0 = work.tile([1, N], BF16, name="rc0", tag="rc0", bufs=4)
                nc.sync.dma_start(rc0[0:1, sl], rc[64:65, sl])
                nc.gpsimd.partition_broadcast(rc64[0:64, sl], rc0[0:1, sl])
                if ro == 0:
                    nc.vector.tensor_mul(ot[tl][0:64, sl], pv[0:64, :], rc64[0:64, sl])
                else:
                    # DVE lanes cannot shift partitions; stage at base 0 and
                    # DMA into partitions 64..127 of the packed OT tile.
                    nc.vector.tensor_mul(og[:, sl], pv[0:64, :], rc64[0:64, sl])
            if ro != 0:
                nc.sync.dma_start(ot[tl][64:128, :], og[:])
            del pv_halves[hl]
        if h == 15:
            for nh in range(2):
                nc.tensor.matmul(
                    pv15[0:65, nh * NF:(nh + 1) * NF],
                    lhsT=v65[7][:, 15 * 65:16 * 65],
                    rhs=pts[1][7][:, nh * NF:(nh + 1) * NF],
                    start=False, stop=True,
                )
            # Drain-tail fast path: PE is idle here, so broadcast the
            # reciprocal row with a K=1 ones-matmul instead of the Pool
            # engine (saves two ~2us Pool wake latencies on the tail).
            rc = work.tile([P, N], BF16, name="rc", tag="rc", bufs=2)
            rc64 = work.tile([P, N], BF16, name="rc64", tag="rc64", bufs=2)
            og = work.tile([64, N], BF16, name="otg", tag="otg", bufs=2)
            with nc.allow_low_precision(reason="softmax denom bf16 ok"):
                nc.vector.reciprocal(rc[64:65, :], pv15[64:65, :])
            bc = psum.tile([P, N], F32, name="bc15", tag="ps", bufs=2)
            for nh in range(2):
                sl = slice(nh * NF, (nh + 1) * NF)
                nc.tensor.matmul(bc[0:64, sl], lhsT=ones_t[64:65, :],
                                 rhs=rc[64:65, sl], start=True, stop=True)
                nc.vector.tensor_copy(rc64[0:64, sl], bc[0:64, sl])
                nc.vector.tensor_mul(og[:, sl], pv15[0:64, sl], rc64[0:64, sl])
            nc.sync.dma_start(ot[7][64:128, :], og[:])

    # ======================= Output projection =======================
    for mt in range(8):
        ps = ps_tile(f"y{mt}")
        for nh in range(2):
            for c in range(8):
                nc.tensor.matmul(
                    ps[:, nh * NF:(nh + 1) * NF],
                    lhsT=pw_t[c][:, mt * P:(mt + 1) * P],
                    rhs=ot[c][:, nh * NF:(nh + 1) * NF],
                    start=(c == 0), stop=(c == 7),
                )
        for nh in range(2):
            y = work.tile([P, NF], F32, name="y", tag="y", bufs=2)
            nc.scalar.activation(y[:], ps[:, nh * NF:(nh + 1) * NF], AF.Identity,
                                 bias=pb_t[:, mt:mt + 1])
            nc.sync.dma_start(out[mt * P:(mt + 1) * P, nh * NF:(nh + 1) * NF], y[:])


def _prep_inputs(x, attn_mask, qkv_w, proj_w, proj_b):
    """Build the 8 per-core input maps (cores 0-3: branch 0 / x_ori with
    batches 0-3; cores 4-7: branch 1 / x_v)."""
    bf = ml_dtypes.bfloat16
    scale = D ** (-0.5)
    q_w, k_w, v_w = qkv_w[0:C], qkv_w[C:2 * C], qkv_w[2 * C:3 * C]
    wT_br = [
        np.ascontiguousarray(np.vstack([q_w * scale, k_w, v_w]).T.astype(bf)),
        np.ascontiguousarray(np.vstack([v_w * scale, v_w, v_w]).T.astype(bf)),
    ]
    em_br = [np.ascontiguousarray(np.exp(attn_mask[br, 0]).T.astype(bf))
             for br in range(2)]
    pwT = np.ascontiguousarray(proj_w.T.astype(bf))
    pb = np.ascontiguousarray(proj_b.astype(np.float32))
    in_maps = []
    for core in range(8):
        br, b = core // 4, core % 4
        in_maps.append({
            "xT": np.ascontiguousarray(x[b].T.astype(bf)),
            "wT": wT_br[br],
            "em": em_br[br],
            "pwT": pwT,
            "pb": pb,
        })
    return in_maps


def _run(inputs, trace=False, **kw):
    global _nc_cache
    from concourse.bass_utils import run_bass_kernel_spmd
    if _nc_cache is None:
        _nc_cache = _build()
    in_maps = _prep_inputs(**inputs)
    res = run_bass_kernel_spmd(_nc_cache, in_maps, core_ids=list(range(8)),
                               trace=trace, **kw)
    outs = [np.asarray(res.results[i]["out"], dtype=np.float32).T
            for i in range(8)]
    x_ori = np.stack(outs[0:4])
    x_v = np.stack(outs[4:8])
    return (x_v, x_ori), res


def kernel(x, attn_mask, qkv_w, proj_w, proj_b):
    (x_v, x_ori), _ = _run(dict(x=np.asarray(x), attn_mask=np.asarray(attn_mask),
                                qkv_w=np.asarray(qkv_w), proj_w=np.asarray(proj_w),
                                proj_b=np.asarray(proj_b)))
    return (x_v, x_ori)


# revision 32
# speedup vs baseline: 1.0046x; 1.0046x over previous
"""Fused dual-branch attention kernel for one TRN2 chip (8 NeuronCores).

Problem: x:[4,1024,1024], qkv_w:[3072,1024], proj_w:[1024,1024], proj_b:[1024],
attn_mask:[2,1,1024,1024].  Reference computes two attention branches sharing
the qkv/proj weights:
  x_ori = proj(attend(q, k, v, mask0)),  x_v = proj(attend(v, v, v, mask1))

Sharding: 8 cores = (2 branches x 4 batches), zero communication.  Every core
runs the SAME graph; branch differences are folded into the per-core weight
data (branch-1 cores get [v_w*s | v_w | v_w] as their "qkv" weight stack) and
the per-core mask data.  The softmax scale folds into the query weights; the
additive mask folds in multiplicatively as exp(mask) (no max-subtraction:
logits are bounded ~+-8 for this distribution, exp stays in fp32 range).

Device layouts (host pre-transposes; all matmul operands land naturally):
  xT   [C, N]      x[b]^T
  wT   [C, 3C]     [A|B|C] weight stack transposed (A = query role, scaled)
  em   [N, N]      exp(mask)^T  (indexed [m, nq])
  pwT  [C, C]      proj_w^T
  out  [C, N]      y^T (host transposes back)

Per-head attention with everything transposed:
  ST[m,nq]  = sum_d BT[d,m] * AT[d,nq]          (K=64 matmul, psum)
  PT[m,nq]  = exp(ST) * em[m,nq]                (ACT exp -> DVE mult, bf16)
  PV        = [V_h | ones]^T-style lhsT gives OT'[d,nq] rows 0..63 and the
              softmax denominator sum_m PT[m,nq] in row 64 of the same psum.
  OT[d,nq]  = OT'[d,nq] * recip(denom)[nq]      (recip bcast across partitions)
Then yT = pwT^T @ OT_all + b.
"""

import numpy as np
import ml_dtypes

import concourse.bass as bass
from concourse import bacc
import concourse.tile as tile
import concourse.mybir as mybir
from contextlib import ExitStack

B, N, C, H, D, P, NF = 4, 1024, 1024, 16, 64, 128, 512
BF16 = mybir.dt.bfloat16
F32 = mybir.dt.float32
AF = mybir.ActivationFunctionType

_nc_cache = None


def _build(reps=1):
    nc = bacc.Bacc("TRN2", target_bir_lowering=False, debug=False, num_devices=8)
    xT = nc.declare_dram_parameter("xT", [C, N], BF16, isOutput=False)
    wT = nc.declare_dram_parameter("wT", [C, 3 * C], BF16, isOutput=False)
    em = nc.declare_dram_parameter("em", [N, N], BF16, isOutput=False)
    pwT = nc.declare_dram_parameter("pwT", [C, C], BF16, isOutput=False)
    pb = nc.declare_dram_parameter("pb", [C], F32, isOutput=False)
    out = nc.declare_dram_parameter("out", [C, N], F32, isOutput=True)

    with tile.TileContext(nc) as tc:
        for _ in range(reps):
            with ExitStack() as ctx:
                _body(tc, ctx, xT, wT, em, pwT, pb, out)
    nc.compile()
    return nc


def _body(tc, ctx, xT, wT, em, pwT, pb, out):
    nc = tc.nc

    pers = ctx.enter_context(tc.tile_pool(name="pers", bufs=1))
    work = ctx.enter_context(tc.tile_pool(name="work", bufs=1))
    psum = ctx.enter_context(tc.tile_pool(name="psum", bufs=1, space="PSUM"))

    def ps_tile(nm):
        return psum.tile([P, N], F32, name=nm, tag="ps", bufs=2)

    # x tiles first (A2 needs them), then the C-role weight columns, then the
    # A/B-role weight columns, then everything else — DMA arrival order
    # matches compute order so PE starts as early as possible.
    xw = ctx.enter_context(tc.tile_pool(name="xw", bufs=1))
    x_t, w_t = [], []
    with tc.tile_pool(name="wc", bufs=1) as wcp:
        wc_t = []
        for c in range(8):
            x = xw.tile([P, N], BF16, name=f"x{c}", tag=f"x{c}")
            nc.sync.dma_start(x[:, 0:P], xT[c * P:(c + 1) * P, 0:P])
            x_t.append(x)
            w = wcp.tile([P, C], BF16, name=f"wc{c}", tag=f"wc{c}")
            nc.sync.dma_start(w[:, 0:NF], wT[c * P:(c + 1) * P, 2 * C:2 * C + NF])
            wc_t.append(w)
        for c in range(8):
            nc.sync.dma_start(x_t[c][:, P:NF], xT[c * P:(c + 1) * P, P:NF])
        for c in range(8):
            nc.sync.dma_start(x_t[c][:, NF:N], xT[c * P:(c + 1) * P, NF:N])
            nc.sync.dma_start(wc_t[c][:, NF:N], wT[c * P:(c + 1) * P, 2 * C + NF:3 * C])
        for c in range(8):
            w = xw.tile([P, 2 * C], BF16, name=f"w{c}", tag=f"w{c}")
            nc.sync.dma_start(w[:], wT[c * P:(c + 1) * P, 0:2 * C])
            w_t.append(w)
        em_t = []
        for m in range(8):
            t = pers.tile([P, N], BF16, name=f"em{m}", tag=f"em{m}")
            nc.sync.dma_start(t[:], em[m * P:(m + 1) * P, :])
            em_t.append(t)
        pb_t = pers.tile([P, 8], F32, name="pb", tag="pb")
        nc.sync.dma_start(pb_t[:], pb.rearrange("(t p) -> p t", p=P))

        ones_t = pers.tile([P, 64], BF16, name="ones", tag="ones")
        nc.vector.memset(ones_t[:], 1.0)
        abt = [pers.tile([P, N], BF16, name=f"abt{i}", tag=f"abt{i}") for i in range(16)]
        v65 = [pers.tile([P, 16 * 65], BF16, name=f"v65_{m}", tag=f"v65_{m}") for m in range(8)]

        # ===== A2 first: V in natural [n, c] layout (+ ones cols at stride 65).
        # vf-outer so the first sweep only needs the first wc halves; the m-th
        # group only needs columns m*128 of each x tile, matching DMA arrival.
        for m in range(8):
            nc.vector.memset(v65[m].rearrange("p (h e) -> p h e", e=65)[:, :, 64:65], 1.0)
        for vf in range(2):
            for m in range(8):
                ps = psum.tile([P, NF], F32, name=f"v{m}_{vf}", tag="qk", bufs=2)
                for c in range(8):
                    nc.tensor.matmul(
                        ps[:],
                        lhsT=x_t[c][:, m * P:(m + 1) * P],
                        rhs=wc_t[c][:, vf * NF:(vf + 1) * NF],
                        start=(c == 0), stop=(c == 7),
                    )
                v3d = v65[m].rearrange("p (h e) -> p h e", e=65)
                nc.vector.tensor_copy(v3d[:, vf * 8:(vf + 1) * 8, 0:64],
                                      ps.rearrange("p (h d) -> p h d", d=64))

    # pools first used after A2 can reuse the wc pool space
    late = ctx.enter_context(tc.tile_pool(name="late", bufs=1))
    pw_t = []
    for c in range(8):
        t = late.tile([P, C], BF16, name=f"pw{c}", tag=f"pw{c}")
        nc.sync.dma_start(t[:], pwT[c * P:(c + 1) * P, :])
        pw_t.append(t)
    ot = [late.tile([P, N], BF16, name=f"ot{i}", tag=f"ot{i}") for i in range(8)]

    # ===== Fused A1 + attention, modulo-scheduled =====
    # Steady state per m-step: PE issues ST(h,m) [2 mm], PV(h-1,m) [2 mm] and
    # every 4th step one A1 half-group [8 mm]; ACT runs exp(h,m); DVE runs the
    # exp-mask multiply, A1 evictions and the normalize chain of head h-1.
    # One-head software pipelining keeps every engine fed; A1 tiles complete
    # ~2 steps before their pair's STs need them.
    def a1_half(mt, nh):
        ps = psum.tile([P, NF], F32, name=f"qk{mt}_{nh}", tag="qk", bufs=2)
        for c in range(8):
            nc.tensor.matmul(
                ps[:],
                lhsT=w_t[c][:, mt * P:(mt + 1) * P],
                rhs=x_t[c][:, nh * NF:(nh + 1) * NF],
                start=(c == 0), stop=(c == 7),
            )
        nc.vector.tensor_copy(abt[mt][:, nh * NF:(nh + 1) * NF], ps[:])

    pts = [[None] * 8, [None] * 8]
    for mt in (0, 8, 1, 9):           # kickstart: q/k tiles for pairs 0 and 1
        for nh in range(2):
            a1_half(mt, nh)
    _a1 = [(8 * r + p, nh) for p in range(2, 8) for r in range(2) for nh in range(2)]
    # Emission step targets: as late as possible while completing pair p's
    # tiles well before its first head; spread ~5 steps apart so late heads
    # keep PE filler work.
    a1q = [(min(5 * i + 3, 16 * (2 + i // 4) - 10 + (i % 4) * 2), mt, nh)
           for i, (mt, nh) in enumerate(_a1)]
    a1q.reverse()                      # pop() takes the earliest target

    pv_halves = {}
    pv15 = None
    for h in range(16):
        # ST / exp / mult for head h, PV for head h-1, paced A1 prefetch
        if h < 16:
            tl, ro = h // 2, (h % 2) * 64
            q_ap, k_ap = abt[tl], abt[8 + tl]
        if h >= 1:
            hl = h - 1
            pvA = psum.tile([P, NF], F32, name=f"pv{hl}a", tag="pv", bufs=2)
            pvB = psum.tile([P, NF], F32, name=f"pv{hl}b", tag="pv", bufs=2)
            pv_halves[hl] = (pvA, pvB)
        for m in range(8):
            if h < 16:
                pt = work.tile([P, N], BF16, name=f"pt{m}", tag=f"pt{m}", bufs=2)
                if h == 15:
                    # Last head: use the narrow qk slots (free once the A1
                    # queue drained) so pv15's drain shortcut gets the wide
                    # ps slots without contention.
                    for nh in range(2):
                        psn = psum.tile([P, NF], F32, name=f"st15_{nh}", tag="qk", bufs=2)
                        nc.tensor.matmul(
                            psn[:],
                            lhsT=k_ap[ro:ro + 64, m * P:(m + 1) * P],
                            rhs=q_ap[ro:ro + 64, nh * NF:(nh + 1) * NF],
                            start=True, stop=True,
                        )
                        e = work.tile([P, NF], BF16, name="esn", tag="est", bufs=3)
                        nc.scalar.activation(e[:], psn[:], AF.Exp)
                        nc.vector.tensor_mul(pt[:, nh * NF:(nh + 1) * NF], e[:],
                                             em_t[m][:, nh * NF:(nh + 1) * NF])
                else:
                    ps = psum.tile([P, N], F32, name=f"st{m}", tag="ps", bufs=2)
                    for nh in range(2):
                        nc.tensor.matmul(
                            ps[:, nh * NF:(nh + 1) * NF],
                            lhsT=k_ap[ro:ro + 64, m * P:(m + 1) * P],
                            rhs=q_ap[ro:ro + 64, nh * NF:(nh + 1) * NF],
                            start=True, stop=True,
                        )
                    e = work.tile([P, N], BF16, name="est", tag="est", bufs=3)
                    nc.scalar.activation(e[:], ps[:], AF.Exp)
                    nc.vector.tensor_mul(pt[:], e[:], em_t[m][:])
                pts[h & 1][m] = pt
            if h >= 1:
                hl = h - 1
                for nh, pv in enumerate(pv_halves[hl]):
                    nc.tensor.matmul(
                        pv[0:65, :],
                        lhsT=v65[m][:, hl * 65:(hl + 1) * 65],
                        rhs=pts[hl & 1][m][:, nh * NF:(nh + 1) * NF],
                        start=(m == 0), stop=(m == 7),
                    )
            if h == 15 and m >= 1:
                # drain shortcut: head 15's PV runs with a 1-chunk lag inside
                # its own step (into a wide ps slot) so the final normalize
                # chain starts ~4us earlier.
                if pv15 is None:
                    pv15 = psum.tile([P, N], F32, name="pv15", tag="ps", bufs=2)
                for nh in range(2):
                    nc.tensor.matmul(
                        pv15[0:65, nh * NF:(nh + 1) * NF],
                        lhsT=v65[m - 1][:, 15 * 65:16 * 65],
                        rhs=pts[1][m - 1][:, nh * NF:(nh + 1) * NF],
                        start=(m == 1), stop=False,
                    )
            gstep = h * 8 + m
            if a1q and gstep >= a1q[-1][0]:
                _, mt, nh = a1q.pop()
                a1_half(mt, nh)
        if h >= 1:
            hl = h - 1
            tl, ro = hl // 2, (hl % 2) * 64
            rc = work.tile([P, N], BF16, name="rc", tag="rc", bufs=2)
            rc64 = work.tile([P, N], BF16, name="rc64", tag="rc64", bufs=2)
            og = (None if ro == 0 else
                  work.tile([64, N], BF16, name="otg", tag="otg", bufs=2))
            for nh, pv in enumerate(pv_halves[hl]):
                sl = slice(nh * NF, (nh + 1) * NF)
                with nc.allow_low_precision(reason="softmax denom bf16 ok"):
                    nc.vector.reciprocal(rc[64:65, sl], pv[64:65, :])
                # HW partition_broadcast reads absolute partition 0 of its
                # input (AP partition offsets are ignored, unlike CoreSim) —
                # DMA the row down to partition 0 first.
                rc0 = work.tile([1, N], BF16, name="rc0", tag="rc0", bufs=4)
                nc.sync.dma_start(rc0[0:1, sl], rc[64:65, sl])
                nc.gpsimd.partition_broadcast(rc64[0:64, sl], rc0[0:1, sl])
                if ro == 0:
                    nc.vector.tensor_mul(ot[tl][0:64, sl], pv[0:64, :], rc64[0:64, sl])
                else:
                    # DVE lanes cannot shift partitions; stage at base 0 and
                    # DMA into partitions 64..127 of the packed OT tile.
                    nc.vector.tensor_mul(og[:, sl], pv[0:64, :], rc64[0:64, sl])
            if ro != 0:
                nc.sync.dma_start(ot[tl][64:128, :], og[:])
            del pv_halves[hl]
        if h == 15:
            for nh in range(2):
                nc.tensor.matmul(
                    pv15[0:65, nh * NF:(nh + 1) * NF],
                    lhsT=v65[7][:, 15 * 65:16 * 65],
                    rhs=pts[1][7][:, nh * NF:(nh + 1) * NF],
                    start=False, stop=True,
                )
            # Drain-tail fast path: PE is idle here, so broadcast the
            # reciprocal row with a K=1 ones-matmul instead of the Pool
            # engine (saves two ~2us Pool wake latencies on the tail).
            rc = work.tile([P, N], BF16, name="rc", tag="rc", bufs=2)
            rc64 = work.tile([P, N], BF16, name="rc64", tag="rc64", bufs=2)
            og = work.tile([64, N], BF16, name="otg", tag="otg", bufs=2)
            with nc.allow_low_precision(reason="softmax denom bf16 ok"):
                nc.vector.reciprocal(rc[64:65, :], pv15[64:65, :])
            bc = psum.tile([P, N], F32, name="bc15", tag="ps", bufs=2)
            for nh in range(2):
                sl = slice(nh * NF, (nh + 1) * NF)
                nc.tensor.matmul(bc[0:64, sl], lhsT=ones_t[64:65, :],
                                 rhs=rc[64:65, sl], start=True, stop=True)
                nc.vector.tensor_copy(rc64[0:64, sl], bc[0:64, sl])
                nc.vector.tensor_mul(og[:, sl], pv15[0:64, sl], rc64[0:64, sl])
            nc.sync.dma_start(ot[7][64:128, :], og[:])

    # ======================= Output projection =======================
    for mt in range(8):
        ps = ps_tile(f"y{mt}")
        for nh in range(2):
            for c in range(8):
                nc.tensor.matmul(
                    ps[:, nh * NF:(nh + 1) * NF],
                    lhsT=pw_t[c][:, mt * P:(mt + 1) * P],
                    rhs=ot[c][:, nh * NF:(nh + 1) * NF],
                    start=(c == 0), stop=(c == 7),
                )
        for nh in range(2):
            y = work.tile([P, NF], F32, name="y", tag="y", bufs=2)
            nc.scalar.activation(y[:], ps[:, nh * NF:(nh + 1) * NF], AF.Identity,
                                 bias=pb_t[:, mt:mt + 1])
            nc.sync.dma_start(out[mt * P:(mt + 1) * P, nh * NF:(nh + 1) * NF], y[:])


def _prep_inputs(x, attn_mask, qkv_w, proj_w, proj_b):
    """Build the 8 per-core input maps (cores 0-3: branch 0 / x_ori with
    batches 0-3; cores 4-7: branch 1 / x_v)."""
    bf = ml_dtypes.bfloat16
    scale = D ** (-0.5)
    q_w, k_w, v_w = qkv_w[0:C], qkv_w[C:2 * C], qkv_w[2 * C:3 * C]
    wT_br = [
        np.ascontiguousarray(np.vstack([q_w * scale, k_w, v_w]).T.astype(bf)),
        np.ascontiguousarray(np.vstack([v_w * scale, v_w, v_w]).T.astype(bf)),
    ]
    em_br = [np.ascontiguousarray(np.exp(attn_mask[br, 0]).T.astype(bf))
             for br in range(2)]
    pwT = np.ascontiguousarray(proj_w.T.astype(bf))
    pb = np.ascontiguousarray(proj_b.astype(np.float32))
    in_maps = []
    for core in range(8):
        br, b = core // 4, core % 4
        in_maps.append({
            "xT": np.ascontiguousarray(x[b].T.astype(bf)),
            "wT": wT_br[br],
            "em": em_br[br],
            "pwT": pwT,
            "pb": pb,
        })
    return in_maps


def _run(inputs, trace=False, **kw):
    global _nc_cache
    from concourse.bass_utils import run_bass_kernel_spmd
    if _nc_cache is None:
        _nc_cache = _build()
    in_maps = _prep_inputs(**inputs)
    res = run_bass_kernel_spmd(_nc_cache, in_maps, core_ids=list(range(8)),
                               trace=trace, **kw)
    outs = [np.asarray(res.results[i]["out"], dtype=np.float32).T
            for i in range(8)]
    x_ori = np.stack(outs[0:4])
    x_v = np.stack(outs[4:8])
    return (x_v, x_ori), res


def kernel(x, attn_mask, qkv_w, proj_w, proj_b):
    (x_v, x_ori), _ = _run(dict(x=np.asarray(x), attn_mask=np.asarray(attn_mask),
                                qkv_w=np.asarray(qkv_w), proj_w=np.asarray(proj_w),
                                proj_b=np.asarray(proj_b)))
    return (x_v, x_ori)
